# revision 9
# baseline (speedup 1.0000x reference)
"""CGCNNConv fused kernel for 8x Trainium2 NeuronCores.

Strategy (edge-parallel, owner-sorted):
- Edges are assigned to the core that owns their src node (node range shard),
  sorted by src, grouped into 49 windows of 128 nodes, each window padded to
  T_W=17 tiles of 128 edges (dummy edges hit all-zero table rows and a
  non-matching one-hot row, so they contribute exactly zero).
- Per core, on device:
  Phase 1: build fp16 projection tables via PE matmuls from host-provided h^T:
     S_loc[n, 0:256]  = [-(h Wg_src) - gb | h Wc_src + cb]   (local 6250 rows)
     T_lo / T_hi[n, :] = [-(h Wg_dst) | h Wc_dst]            (25000 rows each)
  Phase 2: per window: dma_gather rows S_loc[src], T_lo[dst], T_hi[dst]
     (lo/hi split keeps indices within int16; misses hit a zero row),
     preact = gather_S + gather_Tlo + gather_Thi + ef^T @ W_ef  (PE identity-
     add matmuls accumulate everything in PSUM),
     gate (negated) and cand halves through exp/ln-only activations:
       E = exp(preact); U = ln(1+E); G = exp(-U_gate); m = G * U_cand
     scatter-add via one-hot matmul into the window's PSUM accumulator.
  Phase 3: BN stats (partial sums -> 1KB AllReduce), scale/shift, residual,
     softplus, write the core's output slice.
- Host assembles the 8 output slices.
"""

import numpy as np

N_NODES = 50000
N_EDGES = 800000
D = 128
DE = 10
NCORES = 8
NB = N_NODES // NCORES          # 6250 nodes per core
NW = 49                         # windows of 128 nodes (49*128 = 6272 >= 6250)
TW = 18                         # tiles of 128 edges per window
WEDGE = TW * 128                # 2176 edges per window
E_PAD = NW * WEDGE              # 106624 padded edges per core
NLO = 25000                     # T table split point
TROWS = 196 * 128               # 25088 rows per T table (>= NLO + zero row)
SROWS = NW * 128                # 6272 rows in local S table
BN_EPS = 1e-5


def _wrap_idx(flat16):
    """dma_gather index layout: flat[k] -> partition k%16 (replicated x8), free k//16."""
    n = flat16.shape[0]
    arr = flat16.reshape(n // 16, 16).T          # [16, n/16]
    return np.tile(arr, (8, 1))                  # [128, n/16] int16


def _prep_core(k, src, dst, ef_t):
    """Build one core's padded edge ordering + gather/scatter index arrays."""
    base = k * NB
    sel = np.where((src >= base) & (src < base + NB))[0]
    order = np.argsort(src[sel], kind="stable")
    sel = sel[order]
    s_loc = src[sel] - base                      # [Ek] in [0, NB)
    d_glob = dst[sel]

    sgi = np.full(E_PAD, NB, dtype=np.int16)     # S zero row = NB (6250)
    tlo = np.full(E_PAD, NLO, dtype=np.int16)    # T zero rows = 25000
    thi = np.full(E_PAD, NLO, dtype=np.int16)
    wloc = np.full(E_PAD, -512.0, dtype=np.float16)
    eft_pad = np.zeros((DE, E_PAD), dtype=np.float16)

    win = s_loc // 128
    bounds = np.searchsorted(win, np.arange(NW + 1))
    for w in range(NW):
        lo, hi = bounds[w], bounds[w + 1]
        cnt = hi - lo
        assert cnt <= WEDGE, f"window overflow: core {k} win {w} cnt {cnt}"
        p0 = w * WEDGE
        sgi[p0:p0 + cnt] = s_loc[lo:hi].astype(np.int16)
        dw = d_glob[lo:hi]
        is_lo = dw < NLO
        tlo[p0:p0 + cnt] = np.where(is_lo, dw, NLO).astype(np.int16)
        thi[p0:p0 + cnt] = np.where(is_lo, NLO, dw - NLO).astype(np.int16)
        wloc[p0:p0 + cnt] = (s_loc[lo:hi] - 128 * w).astype(np.float16)
        eft_pad[:, p0:p0 + cnt] = ef_t[:, sel[lo:hi]]

    # per-window wrapped idx tiles, concatenated along free dim
    sgi_w = np.concatenate([_wrap_idx(sgi[w * WEDGE:(w + 1) * WEDGE]) for w in range(NW)], axis=1)
    tlo_w = np.concatenate([_wrap_idx(tlo[w * WEDGE:(w + 1) * WEDGE]) for w in range(NW)], axis=1)
    thi_w = np.concatenate([_wrap_idx(thi[w * WEDGE:(w + 1) * WEDGE]) for w in range(NW)], axis=1)
    # winloc: value for edge (tile t, partition p) at [p, w*TW + t]
    wloc_w = wloc.reshape(NW * TW, 128).T.astype(np.float16)     # [128, NW*TW]
    return (sgi_w.view(np.int32).copy(), tlo_w.view(np.int32).copy(),
            thi_w.view(np.int32).copy(), wloc_w.copy(), eft_pad)


def _build_nc():
    import concourse.bass as bass
    import concourse.bacc as bacc
    import concourse.mybir as mybir
    import concourse.tile as tile
    from concourse.masks import make_identity

    f16, f32, i32, i16 = (mybir.dt.float16, mybir.dt.float32,
                          mybir.dt.int32, mybir.dt.int16)
    AF = mybir.ActivationFunctionType
    OP = mybir.AluOpType
    P = 128

    nc = bacc.Bacc("TRN2", target_bir_lowering=False, debug=False,
                   num_devices=NCORES)

    hT = nc.dram_tensor("hT", [P, 50176], f16, kind="ExternalInput")
    hTs = nc.dram_tensor("hTs", [P, SROWS], f16, kind="ExternalInput")
    wsrc = nc.dram_tensor("wsrc", [P, 256], f16, kind="ExternalInput")
    wdst = nc.dram_tensor("wdst", [P, 256], f16, kind="ExternalInput")
    wef = nc.dram_tensor("wef", [DE, 256], f16, kind="ExternalInput")
    bias = nc.dram_tensor("bias", [P, 256], f32, kind="ExternalInput")
    eft = nc.dram_tensor("eft", [DE, E_PAD], f16, kind="ExternalInput")
    sgi = nc.dram_tensor("sgi", [P, NW * WEDGE // 32], i32, kind="ExternalInput")
    tloi = nc.dram_tensor("tloi", [P, NW * WEDGE // 32], i32, kind="ExternalInput")
    thii = nc.dram_tensor("thii", [P, NW * WEDGE // 32], i32, kind="ExternalInput")
    wlocd = nc.dram_tensor("wloc", [P, NW * TW], f16, kind="ExternalInput")
    hres = nc.dram_tensor("hres", [SROWS, D], f32, kind="ExternalInput")
    bng = nc.dram_tensor("bng", [1, D], f32, kind="ExternalInput")
    bnb = nc.dram_tensor("bnb", [1, D], f32, kind="ExternalInput")
    out_d = nc.dram_tensor("out", [SROWS, D], f32, kind="ExternalOutput")

    IW = WEDGE // 32   # int32 cols per window of idx input (68)

    with tile.TileContext(nc) as tc:
        with (
            tc.tile_pool(name="const", bufs=1) as cp,
            tc.tile_pool(name="tabl", bufs=3) as tp,
            tc.tile_pool(name="edge", bufs=2) as ep,
            tc.tile_pool(name="act", bufs=3) as ap_,
            tc.tile_pool(name="psA", bufs=3, space="PSUM") as ppa,
            tc.tile_pool(name="psB", bufs=2, space="PSUM") as ppb,
            tc.tile_pool(name="dram", bufs=1, space="DRAM") as dp,
        ):
            # ---------- constants ----------
            ident = cp.tile([P, P], f16)
            make_identity(nc, ident[:])
            iota_i = cp.tile([P, P], i16)
            nc.gpsimd.iota(iota_i[:], pattern=[[1, P]], base=0, channel_multiplier=0)
            iota_f = cp.tile([P, P], f16)
            nc.vector.tensor_copy(iota_f[:], iota_i[:])
            ones_c = cp.tile([P, 1], f32)
            nc.vector.memset(ones_c[:], 1.0)

            wsrc_s = cp.tile([P, 256], f16)
            nc.sync.dma_start(wsrc_s[:], wsrc[:])
            wdst_s = cp.tile([P, 256], f16)
            nc.sync.dma_start(wdst_s[:], wdst[:])
            wef_s = cp.tile([DE, 256], f16)
            nc.sync.dma_start(wef_s[:], wef[:])
            bias_s = cp.tile([P, 256], f32)
            nc.sync.dma_start(bias_s[:], bias[:])
            zrow = cp.tile([1, 256], f16)
            nc.vector.memset(zrow[:], 0.0)

            # ---------- phase 1: tables ----------
            s_tab = dp.tile([SROWS, 256], f16)
            tlo_tab = dp.tile([TROWS, 256], f16)
            thi_tab = dp.tile([TROWS, 256], f16)

            for i in range(NW):  # S local table, bias folded
                htile = tp.tile([P, P], f16, tag="htile")
                nc.sync.dma_start(htile[:], hTs[:, i * P:(i + 1) * P])
                ps = ppb.tile([P, 256], f32, tag="genps")
                nc.tensor.matmul(ps[:], lhsT=htile[:], rhs=wsrc_s[:], start=True, stop=True)
                row = tp.tile([P, 256], f16, tag="srow")
                nc.vector.tensor_tensor(row[:], ps[:], bias_s[:], op=OP.add)
                nc.sync.dma_start(s_tab[i * P:(i + 1) * P, :], row[:])
            nc.sync.dma_start(s_tab[NB:NB + 1, :], zrow[:])

            for half, tab in ((0, tlo_tab), (1, thi_tab)):
                for i in range(TROWS // P):
                    htile = tp.tile([P, P], f16, tag="htile")
                    nc.sync.dma_start(htile[:], hT[:, half * NLO + i * P: half * NLO + (i + 1) * P])
                    ps = ppb.tile([P, 256], f32, tag="genps")
                    nc.tensor.matmul(ps[:], lhsT=htile[:], rhs=wdst_s[:], start=True, stop=True)
                    row = tp.tile([P, 256], f16, tag="srow")
                    if i % 2 == 0:
                        nc.vector.tensor_copy(row[:], ps[:])
                    else:
                        nc.scalar.copy(row[:], ps[:])
                    nc.sync.dma_start(tab[i * P:(i + 1) * P, :], row[:])
            nc.sync.dma_start(tlo_tab[NLO:NLO + 1, :], zrow[:])

            # ---------- phase 2: edges ----------
            agg = cp.tile([P, NW, D], f32)         # [node%128, window, j]
            rstat = cp.tile([P, 256], f32)         # [sum | sumsq] accumulators
            nc.vector.memset(rstat[:], 0.0)

            for w in range(NW):
                i0 = w * (WEDGE // 32)
                si = ep.tile([P, IW], i32, tag="si")
                nc.sync.dma_start(si[:], sgi[:, i0:i0 + IW])
                li = ep.tile([P, IW], i32, tag="li")
                nc.sync.dma_start(li[:], tloi[:, i0:i0 + IW])
                hi = ep.tile([P, IW], i32, tag="hi")
                nc.sync.dma_start(hi[:], thii[:, i0:i0 + IW])
                wl = ep.tile([P, TW], f16, tag="wl")
                nc.sync.dma_start(wl[:], wlocd[:, w * TW:(w + 1) * TW])
                efts = ep.tile([DE, WEDGE], f16, tag="efts")
                nc.sync.dma_start(efts[:], eft[:, w * WEDGE:(w + 1) * WEDGE])

                zs = ep.tile([P, TW, 256], f16, tag="zs")
                nc.gpsimd.dma_gather(zs[:], s_tab[:], si[:].bitcast(i16),
                                     WEDGE, WEDGE, 256, single_packet=False,
                                     queue_num=0)
                zlo = ep.tile([P, TW, 256], f16, tag="zlo")
                nc.gpsimd.dma_gather(zlo[:], tlo_tab[:], li[:].bitcast(i16),
                                     WEDGE, WEDGE, 256, single_packet=False,
                                     queue_num=0)
                zhi = ep.tile([P, TW, 256], f16, tag="zhi")
                nc.gpsimd.dma_gather(zhi[:], thi_tab[:], hi[:].bitcast(i16),
                                     WEDGE, WEDGE, 256, single_packet=False,
                                     queue_num=0)
                zt = ep.tile([P, TW, 256], f16, tag="zt")
                nc.vector.tensor_tensor(zt[:], zlo[:], zhi[:], op=OP.add)

                pw = ppb.tile([P, D], f32, tag="winps")

                for s in range(TW):
                    pp = ppa.tile([P, 256], f32, tag="pp")
                    nc.tensor.matmul(pp[:], lhsT=efts[:, s * P:(s + 1) * P],
                                     rhs=wef_s[:], start=True, stop=False)
                    nc.tensor.matmul(pp[:], lhsT=ident[:], rhs=zs[:, s, :],
                                     start=False, stop=False)
                    nc.tensor.matmul(pp[:], lhsT=ident[:], rhs=zt[:, s, :],
                                     start=False, stop=True)
                    e16 = ap_.tile([P, 256], f16, tag="e16")
                    nc.scalar.activation(e16[:], pp[:], AF.Exp)
                    u16 = ap_.tile([P, 256], f16, tag="u16")
                    nc.scalar.activation(u16[:], e16[:], AF.Ln, bias=1.0)
                    g16 = ap_.tile([P, D], f16, tag="g16")
                    nc.scalar.activation(g16[:], u16[:, 0:D], AF.Exp, scale=-1.0)
                    m16 = ap_.tile([P, D], f16, tag="m16")
                    nc.vector.tensor_tensor(m16[:], g16[:], u16[:, D:256], op=OP.mult)
                    oh = ap_.tile([P, P], f16, tag="oh")
                    nc.vector.tensor_tensor(oh[:], iota_f[:],
                                            wl[:, s:s + 1].to_broadcast([P, P]),
                                            op=OP.is_equal)
                    nc.tensor.matmul(pw[:], lhsT=oh[:], rhs=m16[:],
                                     start=(s == 0), stop=(s == TW - 1))

                nc.vector.tensor_copy(agg[:, w, :], pw[:])
                sq = ap_.tile([P, D], f32, tag="sq")
                nc.vector.tensor_tensor(sq[:], agg[:, w, :], agg[:, w, :], op=OP.mult)
                nc.vector.tensor_tensor(rstat[:, 0:D], rstat[:, 0:D], agg[:, w, :], op=OP.add)
                nc.vector.tensor_tensor(rstat[:, D:256], rstat[:, D:256], sq[:], op=OP.add)

            # ---------- phase 3: BN stats + output ----------
            pstat = ppb.tile([1, 256], f32, tag="genps")
            nc.tensor.matmul(pstat[:], lhsT=ones_c[:], rhs=rstat[:], start=True, stop=True)
            stat_l = cp.tile([1, 256], f32)
            nc.vector.tensor_copy(stat_l[:], pstat[:])

            cc_in = dp.tile([1, 256], f32)
            cc_out = dp.tile([1, 256], f32)
            nc.gpsimd.dma_start(cc_in[:], stat_l[:])
            nc.gpsimd.collective_compute(
                "AllReduce", OP.add,
                replica_groups=[list(range(NCORES))],
                ins=[cc_in.opt()], outs=[cc_out.opt()])
            stat_g = cp.tile([1, 256], f32)
            nc.sync.dma_start(stat_g[:], cc_out[:])

            bng_s = cp.tile([1, D], f32)
            nc.sync.dma_start(bng_s[:], bng[:])
            bnb_s = cp.tile([1, D], f32)
            nc.sync.dma_start(bnb_s[:], bnb[:])

            mean = cp.tile([1, D], f32)
            nc.vector.tensor_scalar_mul(mean[:], stat_g[:, 0:D], 1.0 / N_NODES)
            ex2 = cp.tile([1, D], f32)
            nc.vector.tensor_scalar_mul(ex2[:], stat_g[:, D:256], 1.0 / N_NODES)
            msq = cp.tile([1, D], f32)
            nc.vector.tensor_tensor(msq[:], mean[:], mean[:], op=OP.mult)
            var = cp.tile([1, D], f32)
            nc.vector.tensor_tensor(var[:], ex2[:], msq[:], op=OP.subtract)
            vpe = cp.tile([1, D], f32)
            nc.vector.tensor_scalar_add(vpe[:], var[:], BN_EPS)
            lnv = cp.tile([1, D], f32)
            nc.scalar.activation(lnv[:], vpe[:], AF.Ln)
            rstd = cp.tile([1, D], f32)
            nc.scalar.activation(rstd[:], lnv[:], AF.Exp, scale=-0.5)
            scale_r = cp.tile([1, D], f32)
            nc.vector.tensor_tensor(scale_r[:], bng_s[:], rstd[:], op=OP.mult)
            mscl = cp.tile([1, D], f32)
            nc.vector.tensor_tensor(mscl[:], mean[:], scale_r[:], op=OP.mult)
            shift_r = cp.tile([1, D], f32)
            nc.vector.tensor_tensor(shift_r[:], bnb_s[:], mscl[:], op=OP.subtract)

            sc_t = cp.tile([P, D], f32)
            nc.gpsimd.partition_broadcast(sc_t[:], scale_r[:])
            sh_t = cp.tile([P, D], f32)
            nc.gpsimd.partition_broadcast(sh_t[:], shift_r[:])

            for w in range(NW):
                ht = tp.tile([P, D], f32, tag="hrt")
                nc.sync.dma_start(ht[:], hres[w * P:(w + 1) * P, :])
                t1 = tp.tile([P, D], f32, tag="t1")
                nc.vector.tensor_tensor(t1[:], agg[:, w, :], sc_t[:], op=OP.mult)
                nc.vector.tensor_tensor(t1[:], t1[:], sh_t[:], op=OP.add)
                nc.vector.tensor_tensor(t1[:], t1[:], ht[:], op=OP.add)
                t2 = tp.tile([P, D], f32, tag="t2")
                nc.scalar.activation(t2[:], t1[:], AF.Exp)
                t3 = tp.tile([P, D], f32, tag="t3")
                nc.scalar.activation(t3[:], t2[:], AF.Ln, bias=1.0)
                nc.sync.dma_start(out_d[w * P:(w + 1) * P, :], t3[:])

    nc.compile()
    return nc


_NC_CACHE = None


def kernel(h, edge_index, edge_feat, gate_w, gate_b, cand_w, cand_b,
           bn_gamma, bn_beta):
    global _NC_CACHE
    from concourse.bass_utils import run_bass_kernel_spmd

    h = np.asarray(h, dtype=np.float32)
    ei = np.asarray(edge_index)
    src = ei[0].astype(np.int64)
    dst = ei[1].astype(np.int64)
    ef = np.asarray(edge_feat, dtype=np.float32)
    gw = np.asarray(gate_w, dtype=np.float32)
    gb = np.asarray(gate_b, dtype=np.float32)
    cw = np.asarray(cand_w, dtype=np.float32)
    cb = np.asarray(cand_b, dtype=np.float32)
    gam = np.asarray(bn_gamma, dtype=np.float32).reshape(1, D)
    bet = np.asarray(bn_beta, dtype=np.float32).reshape(1, D)

    # weight layouts (gate half negated so exp(-a) comes straight from PSUM)
    wsrc = np.concatenate([-gw[0:D], cw[0:D]], axis=1).astype(np.float16)         # [128, 256]
    wdst = np.concatenate([-gw[D:2 * D], cw[D:2 * D]], axis=1).astype(np.float16)
    wef_h = np.concatenate([-gw[2 * D:], cw[2 * D:]], axis=1).astype(np.float16)  # [10, 256]
    bias = np.concatenate([-gb, cb]).astype(np.float32)[None, :].repeat(128, 0)   # [128, 256]

    hT16 = np.zeros((D, 50176), dtype=np.float16)
    hT16[:, :N_NODES] = h.T.astype(np.float16)
    ef_t = ef.T.astype(np.float16)                                                # [10, E]

    in_maps = []
    for k in range(NCORES):
        sgi32, tlo32, thi32, wloc16, eft_pad = _prep_core(k, src, dst, ef_t)
        base = k * NB
        hTs16 = np.zeros((D, SROWS), dtype=np.float16)
        hTs16[:, :NB] = h.T[:, base:base + NB].astype(np.float16)
        hres = np.zeros((SROWS, D), dtype=np.float32)
        hres[:NB] = h[base:base + NB]
        in_maps.append({
            "hT": hT16, "hTs": hTs16, "wsrc": wsrc, "wdst": wdst,
            "wef": wef_h, "bias": bias, "eft": eft_pad,
            "sgi": sgi32, "tloi": tlo32, "thii": thi32, "wloc": wloc16,
            "hres": hres, "bng": gam, "bnb": bet,
        })

    if _NC_CACHE is None:
        _NC_CACHE = _build_nc()
    res = run_bass_kernel_spmd(_NC_CACHE, in_maps, core_ids=list(range(NCORES)))
    out = np.concatenate([res.results[k]["out"][:NB] for k in range(NCORES)], axis=0)
    return out.astype(np.float32)


if __name__ == "__main__":
    import jax
    import reference
    cpu = jax.devices("cpu")[0]
    with jax.default_device(cpu):
        ins = reference.setup_inputs()
        ins = {k: np.asarray(v) for k, v in ins.items()}
        exp = np.asarray(reference.reference(**{k: jax.device_put(v, cpu) for k, v in ins.items()}))
    got = kernel(**ins)
    err = np.abs(got - exp).max() / np.abs(exp).max()
    print("rel err:", err)
